# revision 40
# baseline (speedup 1.0000x reference)
"""Causal multi-head attention (B=2, T=2048, DIM=2048, H=16, HD=128) on 8
Trainium2 NeuronCores.

Sharding: core = 4*b + g  (b = batch 0..1, g = head-group 0..3, 4 heads each).
Each core computes, for its batch b and heads 4g..4g+3:
  QKV projection -> causal attention -> partial out = attn_out @ wo[rows of g]
The host sums the 4 partial outputs per batch (the "all-reduce after wo").

On-device layout avoids every transpose:
  - host passes x[b].T, so projections contract d with d on partitions
  - Q^T/K^T kept as [hd, t] (head dim on partitions): K_h at qkt slot 2h,
    Q_h at slot 2h+1
  - scores computed as S^T = K^T_tile.T @ Q^T  ([j, i] layout)
  - exp via ScalarE; causal masking = multiply diagonal tiles by 0/1 masks
  - diagonal score tiles are NARROWED: only query columns >= key-tile start
    are computed/exp'd/masked (the rest is fully masked anyway)
  - P@V computed as O^T via lhsT = V tile (natural [t, hd] layout)
  - denominator via ones-vector matmul over in-place pair sums on DVE,
    lagged one pair so the add latency stays off the in-order PE
  - normalization via approx reciprocal + GPSIMD partition-broadcast of 1/d
    + VectorE multiply
  - wo projection consumes O^T tiles directly as stationary operands;
    chunks run 0,1,2,3 (chunk 0 is cheapest to run with no interleaved wo
    chains) and every later chunk absorbs the previous chunk's wo chains
    into its score/PV stream
Projection streams d-tiles in groups [2,3,4,3,4] (compute starts after two
d-tiles; group-0 PSUM->SBUF copies alternate DVE/ScalarE so dq=2 stays
PE-bound); per d-tile DMA order is wv, xT, wqk so the first V chain starts
earliest. masks/wo loads ride the scalar HWDGE ring, issued between groups
0 and 1 so they never contend with the first d-tiles or attention.

Everything streams as bfloat16 (ATTN_BF16=0 falls back to float32r): at
MATCHED clock the PE issues bf16 matmuls ~5% faster than f32r (216 vs
227ns) and LDWEIGHTS drops 187->116ns -- the opposite earlier conclusion
was a DVFS artifact (the chip throttles 2.4<->2.0GHz run to run; compare
runs via the DVE op-duration clock reference, never raw times). PSUM,
denominators and the reciprocal stay fp32; output is bf16 (~4.5e-3 rel
err end to end vs the 2e-2 gate). Mixing bf16 with f32r in one matmul is
rejected by the ISA. Further scheduling notes:
  - score/PV/denominator matmuls prefetch 3 tiles ahead; each head's
    denominator pair sums absorb into one running entry on DVE (each
    absorb kills a ones-matmul; the last pair stays separate so the
    flush never waits on a fresh add)
  - the causal mask multiply covers only the 128-wide diagonal strip
  - chain PSUM->SBUF copies run on DVE in-loop (ScalarE is exp-bound),
    alternating engines only in the tail drain
  - GpSimd runs ONLY the 1/d partition-broadcast: any per-tile work on its
    queue (mask muls, pair adds) stalls the PE on Q7 dispatch latency
  - out stores ride the sync ring; the tail drain alternates stores across
    both HWDGE rings
"""

import math
import os

import numpy as np

B, T, D, H, HD = 2, 2048, 2048, 16, 128
NH = 4            # heads per core
NCORES = 8
TCH = 512         # query-chunk width (moving-operand free size)
NDT = D // 128    # 16 d-tiles (contraction tiles for projections)
NTT = T // 128    # 16 t-tiles
NCH = T // TCH    # 4 query chunks

BF16 = os.environ.get("ATTN_BF16", "1") not in ("", "0")

_BUILT = {}
LAST_RESULTS = None  # BassKernelResults of the most recent kernel() call


def _build(causal: bool):
    import concourse.mybir as mybir
    import concourse.tile as tile
    from concourse import bacc

    F32 = mybir.dt.float32
    DT = mybir.dt.bfloat16 if BF16 else mybir.dt.float32r
    EXP = mybir.ActivationFunctionType.Exp
    scale = 1.0 / math.sqrt(HD)

    nc = bacc.Bacc(None, name="attn")
    xT = nc.dram_tensor("xT", [D, T], DT, kind="ExternalInput")
    wqkv = nc.dram_tensor("wqkv", [D, 3 * NH * HD], DT, kind="ExternalInput")
    wo = nc.dram_tensor("wo", [NH * HD, D], DT, kind="ExternalInput")
    masks = nc.dram_tensor("masks", [128, 4 * TCH], DT, kind="ExternalInput")
    if not causal:
        maskT = nc.dram_tensor("maskT", [T, T], DT, kind="ExternalInput")
    BF = mybir.dt.bfloat16
    # output in bf16: halves store traffic and PSUM->SBUF copy cost on
    # the saturated DVE/ScalarE; costs ~1e-3 rel err vs the 2e-2 gate
    out = nc.dram_tensor("out", [T, D], BF, kind="ExternalOutput")

    with tile.TileContext(nc) as tc:
        with (
            tc.tile_pool(name="persist", bufs=1) as persist,
            tc.tile_pool(name="work", bufs=21) as work,
            tc.tile_pool(name="ps3", bufs=3, space="PSUM") as ps3,
        ):
            # persistent operands for the attention phase
            qkt = persist.tile([128, 8, T], DT)           # slot 2h: K_h, 2h+1: Q_h
            vsb = persist.tile([128, NTT, NH * HD], DT)   # V, [t-tile][local t, head*hd]
            msb = persist.tile([128, 4 * TCH], DT)        # diagonal causal masks
            ones_f = persist.tile([128, 1], F32)
            ones = persist.tile([128, 1], DT)
            nc.vector.memset(ones_f[:], 1.0)
            nc.vector.tensor_copy(ones[:], ones_f[:])
            # dummy broadcast: preload the GpSimd PartitionBroadcast ucode
            # library now (~11us HBM fetch) so the first real normalize
            # doesn't stall the whole attention pipeline on LIBRARY_RELOAD
            warm = persist.tile([128, 1], F32)
            nc.gpsimd.partition_broadcast(warm[:], ones_f[0:1, :])

            # ---- Phase A: QKV projections, streaming x^T / wqkv d-tiles.
            # groups[0]=2 so compute starts after ~2.5MB of DMA; its
            # PSUM->SBUF copies split across DVE+ScalarE keep dq=2 PE-bound.
            # Later groups are >=3 d-tiles so the SBUF-accumulate adds (DVE
            # only) stay cheaper than the PE chain time. ----
            groups = [2, 3, 4, 3, 4]
            offs = [sum(groups[:i]) for i in range(len(groups))]
            comb_n = [0]
            pre_pts = {}
            with (
                tc.tile_pool(name="xw", bufs=9) as xw,
                tc.tile_pool(name="pp", bufs=3, space="PSUM") as ppool,
            ):
                for qg, (off, dq) in enumerate(zip(offs, groups)):
                    last = qg == len(groups) - 1
                    xts, wqks, wvs = [], [], []
                    for k in range(dq):
                        di = off + k
                        wv_t = xw.tile([128, NH * HD], DT, tag="wv", bufs=2)
                        xt_t = xw.tile([128, T], DT, tag="xt")
                        wqk_t = xw.tile([128, 2 * NH * HD], DT, tag="wqk", bufs=2)
                        if qg == 0:
                            # group 0's two d-tiles are needed simultaneously:
                            # split them across BOTH HWDGE rings (one ring
                            # sustains only ~170GB/s per in-flight transfer)
                            r0 = nc.sync if k == 0 else nc.scalar
                            r1 = nc.scalar if k == 0 else nc.sync
                            r0.dma_start(wv_t[:],
                                         wqkv[di * 128:(di + 1) * 128,
                                              2 * NH * HD:3 * NH * HD])
                            r0.dma_start(xt_t[:, 0:T // 2],
                                         xT[di * 128:(di + 1) * 128, 0:T // 2])
                            r0.dma_start(xt_t[:, T // 2:T],
                                         xT[di * 128:(di + 1) * 128, T // 2:T])
                            r1.dma_start(wqk_t[:],
                                         wqkv[di * 128:(di + 1) * 128,
                                              0:2 * NH * HD])
                        else:
                            # one combined wqkv DMA per steady-state d-tile:
                            # fewer issues and one less DMA semaphore chain
                            # coupling into the accumulate adds
                            wall_t = xw.tile([128, 3 * NH * HD], DT,
                                             tag="wqkv", bufs=9)
                            nc.sync.dma_start(wall_t[:],
                                              wqkv[di * 128:(di + 1) * 128, :])
                            nc.sync.dma_start(xt_t[:],
                                              xT[di * 128:(di + 1) * 128, :])
                            wv_t = wall_t[:, 2 * NH * HD:3 * NH * HD]
                            wqk_t = wall_t[:, 0:2 * NH * HD]
                        xts.append(xt_t)
                        wqks.append(wqk_t)
                        wvs.append(wv_t)

                    def acc(ps, dst):
                        comb_n[0] += 1
                        if qg == 0:
                            if comb_n[0] % 2 == 0:
                                nc.scalar.copy(dst, ps[:])
                            else:
                                nc.vector.tensor_copy(dst, ps[:])
                        else:
                            nc.vector.tensor_add(dst, dst, ps[:])

                    # V first: attention's PV chains need V earliest
                    for tt in range(NTT):
                        ps = ppool.tile([128, TCH], F32, tag="pp")
                        for k in range(dq):
                            nc.tensor.matmul(
                                ps[:],
                                xts[k][:, tt * 128:(tt + 1) * 128],
                                wvs[k][:],
                                start=(k == 0),
                                stop=(k == dq - 1),
                            )
                        acc(ps, vsb[:, tt, :])
                    # Q^T / K^T chains, chunk-ascending (attention starts
                    # at chunk 0)
                    tchs = (0, 1, 2, 3)
                    for tch in tchs:
                        for h in range(NH):
                            for sl, wof in ((0, NH * HD + h * HD), (1, h * HD)):
                                ps = ppool.tile([128, TCH], F32, tag="pp")
                                for k in range(dq):
                                    nc.tensor.matmul(
                                        ps[:],
                                        wqks[k][:, wof:wof + HD],
                                        xts[k][:, tch * TCH:(tch + 1) * TCH],
                                        start=(k == 0),
                                        stop=(k == dq - 1),
                                    )
                                acc(ps, qkt[:, 2 * h + sl,
                                            tch * TCH:(tch + 1) * TCH])
                        if causal and last and tch <= 1:
                            # pre-emit opening score tiles (all of chunk 0
                            # after its QK chains; chunk 1 head 0's prefetch
                            # window after chunk 1's): the small matmuls
                            # interleave into the remaining QK chains and
                            # their exps/mask-muls retire on the near-idle
                            # ScalarE/DVE during the proj tail, so attention
                            # starts with its pt tiles ready
                            if tch == 0:
                                todo = [(0, h, jt) for h in range(NH)
                                        for jt in range(4)]
                            else:
                                todo = [(1, 0, jt) for jt in range(3)]
                            for (pc, h, jt) in todo:
                                qd = jt - 4 * pc
                                o = qd * 128 if qd > 0 else 0
                                pss = ps3.tile([128, TCH], F32, tag="ps_s")
                                nc.tensor.matmul(
                                    pss[:, o:],
                                    qkt[:, 2 * h, jt * 128:(jt + 1) * 128],
                                    qkt[:, 2 * h + 1,
                                        pc * TCH + o:(pc + 1) * TCH],
                                    start=True,
                                    stop=True,
                                )
                                pt = work.tile([128, TCH], DT, tag="pt")
                                nc.scalar.activation(pt[:, o:], pss[:, o:],
                                                     EXP, scale=scale)
                                if qd >= 0:
                                    nc.vector.tensor_mul(
                                        pt[:, o:o + 128], pt[:, o:o + 128],
                                        msb[:, qd * TCH + o:qd * TCH + o + 128])
                                pre_pts[(pc, h, jt)] = (pt, o)
                    if qg == 0:
                        # masks on the scalar HWDGE ring, issued after group
                        # 0's copies: the transfer lands mid-proj without
                        # contending with the first d-tiles, and the tile is
                        # persistent so nothing waits on SBUF reuse
                        nc.scalar.dma_start(msb[:], masks[:])

            # ---- Phase B+C: attention per chunk, with the previous chunk's
            # wo-projection chains interleaved into the jt loop so the
            # in-order PE never sits on PSUM slot recycling ----
            with (
                tc.tile_pool(name="post", bufs=1) as post,
                tc.tile_pool(name="sml", bufs=2) as sml,
                tc.tile_pool(name="otp", bufs=2) as otp,
                tc.tile_pool(name="outp", bufs=4) as outp,
                tc.tile_pool(name="pso", bufs=4, space="PSUM") as pso4,
                tc.tile_pool(name="psd2", bufs=1, space="PSUM") as ps1,
            ):
                # wo tiles land in the freed xw region; their dma_starts ride
                # the sync ring (idle until the first out-store ~15us later)
                # so the slot-free wait never blocks the scalar exp stream
                wosb = []
                for et in range(NH):
                    wt_ = post.tile([128, D], DT, tag=f"wos{et}")
                    nc.sync.dma_start(wt_[:], wo[et * 128:(et + 1) * 128, :])
                    wosb.append(wt_)

                pc_n = [0]

                def emit_pc_chain(c0, lt, oc, otc0, alternate=False):
                    if alternate and pc_n[0] % 2 == 0:
                        ps = ps3.tile([128, TCH], F32, tag="ps_s")
                    else:
                        ps = pso4.tile([128, TCH], F32, tag="ps_o")
                    for h2 in range(NH):
                        nc.tensor.matmul(
                            ps[:],
                            otc0[:, h2, lt * 128:(lt + 1) * 128],
                            wosb[h2][:, oc * TCH:(oc + 1) * TCH],
                            start=(h2 == 0),
                            stop=(h2 == NH - 1),
                        )
                    ost = outp.tile([128, TCH], BF, tag="ost")
                    pc_n[0] += 1
                    if alternate and pc_n[0] % 2 == 0:
                        nc.scalar.copy(ost[:], ps[:])
                    else:
                        nc.vector.tensor_copy(ost[:], ps[:])
                    deng = nc.scalar if (alternate and pc_n[0] % 2 == 0) \
                        else nc.sync
                    deng.dma_start(
                        out[(4 * c0 + lt) * 128:(4 * c0 + lt + 1) * 128,
                            oc * TCH:(oc + 1) * TCH],
                        ost[:],
                    )

                pending = []
                fin_q = []
                for c in (0, 1, 2, 3):
                    otc = otp.tile([128, NH, TCH], DT, tag="ot")
                    njt = 4 * (c + 1) if causal else NTT
                    steps_total = NH * njt
                    spacing = max(1, steps_total // len(pending)) if pending else 0
                    step = 0
                    for h in range(NH):
                        pso = pso4.tile([128, TCH], F32, tag="ps_o")
                        psd = ps1.tile([1, TCH], F32, tag="ps_d")

                        def emit_pss(jt):
                            # diagonal tiles: queries < 128*qd are fully
                            # masked for this key tile -> narrow the moving
                            # operand and all downstream elementwise work
                            qd = jt - 4 * c
                            o = qd * 128 if (causal and qd > 0) else 0
                            pss = ps3.tile([128, TCH], F32, tag="ps_s")
                            nc.tensor.matmul(
                                pss[:, o:],
                                qkt[:, 2 * h, jt * 128:(jt + 1) * 128],
                                qkt[:, 2 * h + 1, c * TCH + o:(c + 1) * TCH],
                                start=True,
                                stop=True,
                            )
                            pt = work.tile([128, TCH], DT, tag="pt")
                            nc.scalar.activation(pt[:, o:], pss[:, o:], EXP,
                                                 scale=scale)
                            if causal:
                                if qd >= 0:
                                    # the mask only zeroes the 128-wide
                                    # strip at the diagonal (keys span 128
                                    # rows), so the mul covers just that strip
                                    nc.vector.tensor_mul(
                                        pt[:, o:o + 128], pt[:, o:o + 128],
                                        msb[:, qd * TCH + o:qd * TCH + o + 128])
                            else:
                                mt = work.tile([128, TCH], DT, tag="mt")
                                nc.sync.dma_start(
                                    mt[:],
                                    maskT[jt * 128:(jt + 1) * 128,
                                          c * TCH:(c + 1) * TCH],
                                )
                                nc.vector.tensor_mul(pt[:], pt[:], mt[:])
                            return pt, o

                        pts = {}
                        jt0 = 0
                        while causal and (c, h, jt0) in pre_pts:
                            pts[jt0] = pre_pts[(c, h, jt0)]
                            jt0 += 1
                        for jt in range(jt0, min(3, njt)):
                            pts[jt] = emit_pss(jt)
                        prev_pt = prev_off = None
                        dpend = []   # pair sums awaiting their ones-matmul
                        dn = [0]

                        def flush_den(stop, dp_=None, psd_=None, dn_=None):
                            # bind per-head state via defaults at call sites:
                            # the loop rebinds dpend/psd/dn each head, so a
                            # deferred call must not resolve them by closure
                            dp_ = dpend if dp_ is None else dp_
                            psd_ = psd if psd_ is None else psd_
                            dn_ = dn if dn_ is None else dn_
                            fpt, foff, _ = dp_.pop(0)
                            nc.tensor.matmul(
                                psd_[:, foff:],
                                ones[:, 0:1],
                                fpt[:, foff:],
                                start=(dn_[0] == 0),
                                stop=stop,
                            )
                            dn_[0] += 1

                        for jt in range(njt):
                            if jt + 3 < njt and jt + 3 not in pts:
                                pts[jt + 3] = emit_pss(jt + 3)
                            pt, o = pts.pop(jt)
                            nc.tensor.matmul(
                                pso[:, o:],
                                vsb[:, jt, h * HD:(h + 1) * HD],
                                pt[:, o:],
                                start=(jt == 0),
                                stop=(jt == njt - 1),
                            )
                            if jt == 0 and fin_q:
                                fin_q.pop(0)()
                            # denominator: sum pt pairs in place on DVE (the
                            # odd tile's valid region is a suffix of the even
                            # tile's) and run the ones-matmul one pair late so
                            # the DVE add latency never stalls the PE
                            if jt % 2 == 0:
                                prev_pt, prev_off = pt, o
                            else:
                                nc.vector.tensor_add(prev_pt[:, o:],
                                                     prev_pt[:, o:], pt[:, o:])
                                if (dpend and dpend[0][1] <= prev_off
                                        and jt != njt - 1):
                                    # absorb into the head's running entry on
                                    # DVE: each absorb kills one ones-matmul;
                                    # the last pair stays separate so the
                                    # flush never waits on a fresh add
                                    e = dpend[0][0]
                                    nc.vector.tensor_add(
                                        e[:, prev_off:], e[:, prev_off:],
                                        prev_pt[:, prev_off:])
                                else:
                                    dpend.append([prev_pt, prev_off, False])
                            step += 1
                            if pending and spacing and step % spacing == 0:
                                emit_pc_chain(*pending.pop(0))
                        def finalize(dp=dpend, fl=flush_den, ps_o=pso,
                                     ps_d=psd, oc_=otc, h_=h, dn_c=dn):
                            while dp:
                                fl(stop=(len(dp) == 1), dp_=dp, psd_=ps_d,
                                   dn_=dn_c)
                            # 1/d via approx reciprocal (~18 bits, beyond the
                            # bf16 pipeline); exact recip costs 3.3us
                            drc = sml.tile([1, TCH], F32, tag="drc")
                            nc.vector.reciprocal_approx_fast(drc[:], ps_d[:])
                            bc = sml.tile([128, TCH], F32, tag="bc")
                            nc.gpsimd.partition_broadcast(bc[:], drc[:])
                            nc.vector.tensor_mul(oc_[:, h_, :], ps_o[:], bc[:])
                        fin_q.append(finalize)
                    while pending:
                        emit_pc_chain(*pending.pop(0))
                    pending = [(c, lt, oc, otc)
                               for lt in range(4) for oc in range(NCH)]
                while fin_q:
                    fin_q.pop(0)()
                # tail drain: alternate the PSUM->SBUF copies between DVE and
                # ScalarE so slot recycling isn't single-engine-latency-bound
                for chain in pending:
                    emit_pc_chain(*chain, alternate=True)
    nc.compile()
    return nc


def _get_built(causal: bool):
    if causal not in _BUILT:
        _BUILT[causal] = _build(causal)
    return _BUILT[causal]


def _diag_masks():
    # masks[jl, q*TCH + ii] = 1 if key (128*q + jl) <= query ii in the chunk
    q = np.arange(4)[:, None, None]
    jl = np.arange(128)[None, :, None]
    ii = np.arange(TCH)[None, None, :]
    m = (ii >= 128 * q + jl).astype(np.float32)        # [4, 128, TCH]
    return np.ascontiguousarray(m.transpose(1, 0, 2).reshape(128, 4 * TCH))


def kernel(x, mask, wqkv, wo):
    global LAST_RESULTS
    from concourse.bass_utils import run_bass_kernel_spmd

    if BF16:
        import ml_dtypes
        sdt = ml_dtypes.bfloat16
    else:
        sdt = np.float32

    x = np.ascontiguousarray(np.asarray(x, dtype=np.float32))
    wqkv = np.asarray(wqkv, dtype=np.float32)
    wo_f = np.ascontiguousarray(np.asarray(wo, dtype=np.float32).astype(sdt))
    mask_np = np.asarray(mask).reshape(T, T).astype(bool)
    causal = bool(np.array_equal(mask_np, np.tril(np.ones((T, T), dtype=bool))))

    nc = _get_built(causal)
    masks_arr = _diag_masks().astype(sdt)
    maskT = None
    if not causal:
        maskT = np.ascontiguousarray(mask_np.T.astype(sdt))

    in_maps = []
    for core in range(NCORES):
        b, g = divmod(core, NH)
        xT = np.ascontiguousarray(x[b].T.astype(sdt))
        wq = wqkv[:, 0 * H * HD + g * NH * HD:0 * H * HD + (g + 1) * NH * HD]
        wk = wqkv[:, 1 * H * HD + g * NH * HD:1 * H * HD + (g + 1) * NH * HD]
        wv = wqkv[:, 2 * H * HD + g * NH * HD:2 * H * HD + (g + 1) * NH * HD]
        wqkv_g = np.ascontiguousarray(
            np.concatenate([wq, wk, wv], axis=1).astype(sdt))
        wo_g = np.ascontiguousarray(wo_f[g * NH * HD:(g + 1) * NH * HD, :])
        m = {"xT": xT, "wqkv": wqkv_g, "wo": wo_g, "masks": masks_arr}
        if maskT is not None:
            m["maskT"] = maskT
        in_maps.append(m)

    trace = os.environ.get("ATTN_TRACE", "") not in ("", "0")
    res = run_bass_kernel_spmd(nc, in_maps, core_ids=list(range(NCORES)),
                               trace=trace)
    LAST_RESULTS = res

    acc = np.zeros((B, T, D), dtype=np.float64)
    for core in range(NCORES):
        b = core // NH
        acc[b] += res.results[core]["out"].astype(np.float64)
    return acc.astype(np.float32)


# revision 41
# speedup vs baseline: 1.1857x; 1.1857x over previous
"""Causal multi-head attention (B=2, T=2048, DIM=2048, H=16, HD=128) on 8
Trainium2 NeuronCores.

Sharding: core = 4*b + g  (b = batch 0..1, g = head-group 0..3, 4 heads each).
Each core computes, for its batch b and heads 4g..4g+3:
  QKV projection -> causal attention -> partial out = attn_out @ wo[rows of g]
The host sums the 4 partial outputs per batch (the "all-reduce after wo").

On-device layout avoids every transpose:
  - host passes x[b].T, so projections contract d with d on partitions
  - Q^T/K^T kept as [hd, t] (head dim on partitions): K_h at qkt slot 2h,
    Q_h at slot 2h+1
  - scores computed as S^T = K^T_tile.T @ Q^T  ([j, i] layout)
  - exp via ScalarE; causal masking = multiply diagonal tiles by 0/1 masks
  - diagonal score tiles are NARROWED: only query columns >= key-tile start
    are computed/exp'd/masked (the rest is fully masked anyway)
  - P@V computed as O^T via lhsT = V tile (natural [t, hd] layout)
  - denominator via ones-vector matmul over in-place pair sums on DVE,
    lagged one pair so the add latency stays off the in-order PE
  - normalization via approx reciprocal + GPSIMD partition-broadcast of 1/d
    + VectorE multiply
  - wo projection consumes O^T tiles directly as stationary operands;
    chunks run 0,1,2,3 (chunk 0 is cheapest to run with no interleaved wo
    chains) and every later chunk absorbs the previous chunk's wo chains
    into its score/PV stream
Projection streams d-tiles in groups [2,3,4,3,4] (compute starts after two
d-tiles; group-0 PSUM->SBUF copies alternate DVE/ScalarE so dq=2 stays
PE-bound); per d-tile DMA order is wv, xT, wqk so the first V chain starts
earliest. masks/wo loads ride the scalar HWDGE ring, issued between groups
0 and 1 so they never contend with the first d-tiles or attention.

Everything streams as bfloat16 (ATTN_BF16=0 falls back to float32r): at
MATCHED clock the PE issues bf16 matmuls ~5% faster than f32r (216 vs
227ns) and LDWEIGHTS drops 187->116ns -- the opposite earlier conclusion
was a DVFS artifact (the chip throttles 2.4<->2.0GHz run to run; compare
runs via the DVE op-duration clock reference, never raw times). PSUM,
denominators and the reciprocal stay fp32; output is bf16 (~4.5e-3 rel
err end to end vs the 2e-2 gate). Mixing bf16 with f32r in one matmul is
rejected by the ISA. Further scheduling notes:
  - score/PV/denominator matmuls prefetch 3 tiles ahead; each head's
    denominator pair sums absorb into one running entry on DVE (each
    absorb kills a ones-matmul; the last pair stays separate so the
    flush never waits on a fresh add)
  - the causal mask multiply covers only the 128-wide diagonal strip
  - chain PSUM->SBUF copies run on DVE in-loop (ScalarE is exp-bound),
    alternating engines only in the tail drain
  - GpSimd runs ONLY the 1/d partition-broadcast: any per-tile work on its
    queue (mask muls, pair adds) stalls the PE on Q7 dispatch latency
  - out stores ride the sync ring; the tail drain alternates stores across
    both HWDGE rings
"""

import math
import os

import numpy as np

B, T, D, H, HD = 2, 2048, 2048, 16, 128
NH = 4            # heads per core
NCORES = 8
TCH = 512         # query-chunk width (moving-operand free size)
NDT = D // 128    # 16 d-tiles (contraction tiles for projections)
NTT = T // 128    # 16 t-tiles
NCH = T // TCH    # 4 query chunks

BF16 = os.environ.get("ATTN_BF16", "1") not in ("", "0")

_BUILT = {}
LAST_RESULTS = None  # BassKernelResults of the most recent kernel() call


def _build(causal: bool):
    import concourse.mybir as mybir
    import concourse.tile as tile
    from concourse import bacc

    F32 = mybir.dt.float32
    DT = mybir.dt.bfloat16 if BF16 else mybir.dt.float32r
    EXP = mybir.ActivationFunctionType.Exp
    scale = 1.0 / math.sqrt(HD)

    nc = bacc.Bacc(None, name="attn")
    xT = nc.dram_tensor("xT", [D, T], DT, kind="ExternalInput")
    wqkv = nc.dram_tensor("wqkv", [D, 3 * NH * HD], DT, kind="ExternalInput")
    wo = nc.dram_tensor("wo", [NH * HD, D], DT, kind="ExternalInput")
    masks = nc.dram_tensor("masks", [128, 4 * TCH], DT, kind="ExternalInput")
    if not causal:
        maskT = nc.dram_tensor("maskT", [T, T], DT, kind="ExternalInput")
    BF = mybir.dt.bfloat16
    # output in bf16: halves store traffic and PSUM->SBUF copy cost on
    # the saturated DVE/ScalarE; costs ~1e-3 rel err vs the 2e-2 gate
    out = nc.dram_tensor("out", [T, D], BF, kind="ExternalOutput")

    with tile.TileContext(nc) as tc:
        with (
            tc.tile_pool(name="persist", bufs=1) as persist,
            tc.tile_pool(name="work", bufs=21) as work,
            tc.tile_pool(name="ps3", bufs=3, space="PSUM") as ps3,
        ):
            # persistent operands for the attention phase
            qkt = persist.tile([128, 8, T], DT)           # slot 2h: K_h, 2h+1: Q_h
            vsb = persist.tile([128, NTT, NH * HD], DT)   # V, [t-tile][local t, head*hd]
            msb = persist.tile([128, 4 * TCH], DT)        # diagonal causal masks
            ones_f = persist.tile([128, 1], F32)
            ones = persist.tile([128, 1], DT)
            nc.vector.memset(ones_f[:], 1.0)
            nc.vector.tensor_copy(ones[:], ones_f[:])
            # dummy broadcast: preload the GpSimd PartitionBroadcast ucode
            # library now (~11us HBM fetch) so the first real normalize
            # doesn't stall the whole attention pipeline on LIBRARY_RELOAD
            warm = persist.tile([128, 1], F32)
            nc.gpsimd.partition_broadcast(warm[:], ones_f[0:1, :])

            # ---- Phase A: QKV projections, streaming x^T / wqkv d-tiles.
            # groups[0]=2 so compute starts after ~2.5MB of DMA; its
            # PSUM->SBUF copies split across DVE+ScalarE keep dq=2 PE-bound.
            # Later groups are >=3 d-tiles so the SBUF-accumulate adds (DVE
            # only) stay cheaper than the PE chain time. ----
            groups = [2, 3, 4, 3, 4]
            offs = [sum(groups[:i]) for i in range(len(groups))]
            comb_n = [0]
            pre_pts = {}
            with (
                tc.tile_pool(name="xw", bufs=9) as xw,
                tc.tile_pool(name="pp", bufs=3, space="PSUM") as ppool,
            ):
                for qg, (off, dq) in enumerate(zip(offs, groups)):
                    last = qg == len(groups) - 1
                    xts, wqks, wvs = [], [], []
                    for k in range(dq):
                        di = off + k
                        wv_t = xw.tile([128, NH * HD], DT, tag="wv", bufs=2)
                        xt_t = xw.tile([128, T], DT, tag="xt")
                        wqk_t = xw.tile([128, 2 * NH * HD], DT, tag="wqk", bufs=2)
                        if qg == 0:
                            # group 0's two d-tiles are needed simultaneously:
                            # split them across BOTH HWDGE rings (one ring
                            # sustains only ~170GB/s per in-flight transfer)
                            r0 = nc.sync if k == 0 else nc.scalar
                            r1 = nc.scalar if k == 0 else nc.sync
                            r0.dma_start(wv_t[:],
                                         wqkv[di * 128:(di + 1) * 128,
                                              2 * NH * HD:3 * NH * HD])
                            r0.dma_start(xt_t[:, 0:T // 2],
                                         xT[di * 128:(di + 1) * 128, 0:T // 2])
                            r0.dma_start(xt_t[:, T // 2:T],
                                         xT[di * 128:(di + 1) * 128, T // 2:T])
                            r1.dma_start(wqk_t[:],
                                         wqkv[di * 128:(di + 1) * 128,
                                              0:2 * NH * HD])
                        else:
                            # one combined wqkv DMA per steady-state d-tile:
                            # fewer issues and one less DMA semaphore chain
                            # coupling into the accumulate adds
                            wall_t = xw.tile([128, 3 * NH * HD], DT,
                                             tag="wqkv", bufs=9)
                            nc.sync.dma_start(wall_t[:],
                                              wqkv[di * 128:(di + 1) * 128, :])
                            nc.sync.dma_start(xt_t[:],
                                              xT[di * 128:(di + 1) * 128, :])
                            wv_t = wall_t[:, 2 * NH * HD:3 * NH * HD]
                            wqk_t = wall_t[:, 0:2 * NH * HD]
                        xts.append(xt_t)
                        wqks.append(wqk_t)
                        wvs.append(wv_t)

                    def acc(ps, dst):
                        comb_n[0] += 1
                        if qg == 0:
                            if comb_n[0] % 2 == 0:
                                nc.scalar.copy(dst, ps[:])
                            else:
                                nc.vector.tensor_copy(dst, ps[:])
                        else:
                            nc.vector.tensor_add(dst, dst, ps[:])

                    # V first: attention's PV chains need V earliest
                    for tt in range(NTT):
                        ps = ppool.tile([128, TCH], F32, tag="pp")
                        for k in range(dq):
                            nc.tensor.matmul(
                                ps[:],
                                xts[k][:, tt * 128:(tt + 1) * 128],
                                wvs[k][:],
                                start=(k == 0),
                                stop=(k == dq - 1),
                            )
                        acc(ps, vsb[:, tt, :])
                    # Q^T / K^T chains, chunk-ascending (attention starts
                    # at chunk 0)
                    tchs = (0, 1, 2, 3)
                    for tch in tchs:
                        for h in range(NH):
                            for sl, wof in ((0, NH * HD + h * HD), (1, h * HD)):
                                ps = ppool.tile([128, TCH], F32, tag="pp")
                                for k in range(dq):
                                    nc.tensor.matmul(
                                        ps[:],
                                        wqks[k][:, wof:wof + HD],
                                        xts[k][:, tch * TCH:(tch + 1) * TCH],
                                        start=(k == 0),
                                        stop=(k == dq - 1),
                                    )
                                acc(ps, qkt[:, 2 * h + sl,
                                            tch * TCH:(tch + 1) * TCH])
                        if causal and last and tch <= 1:
                            # pre-emit opening score tiles (all of chunk 0
                            # after its QK chains; chunk 1 head 0's prefetch
                            # window after chunk 1's): the small matmuls
                            # interleave into the remaining QK chains and
                            # their exps/mask-muls retire on the near-idle
                            # ScalarE/DVE during the proj tail, so attention
                            # starts with its pt tiles ready
                            if tch == 0:
                                todo = [(0, h, jt) for h in range(NH)
                                        for jt in range(4)]
                            else:
                                todo = [(1, 0, jt) for jt in range(3)]
                            for (pc, h, jt) in todo:
                                qd = jt - 4 * pc
                                o = qd * 128 if qd > 0 else 0
                                pss = ps3.tile([128, TCH], F32, tag="ps_s")
                                nc.tensor.matmul(
                                    pss[:, o:],
                                    qkt[:, 2 * h, jt * 128:(jt + 1) * 128],
                                    qkt[:, 2 * h + 1,
                                        pc * TCH + o:(pc + 1) * TCH],
                                    start=True,
                                    stop=True,
                                )
                                pt = work.tile([128, TCH], DT, tag="pt")
                                nc.scalar.activation(pt[:, o:], pss[:, o:],
                                                     EXP, scale=scale)
                                if qd >= 0:
                                    nc.vector.tensor_mul(
                                        pt[:, o:o + 128], pt[:, o:o + 128],
                                        msb[:, qd * TCH + o:qd * TCH + o + 128])
                                pre_pts[(pc, h, jt)] = (pt, o)
                    if qg == 0:
                        # masks on the scalar HWDGE ring, issued after group
                        # 0's copies: the transfer lands mid-proj without
                        # contending with the first d-tiles, and the tile is
                        # persistent so nothing waits on SBUF reuse
                        nc.scalar.dma_start(msb[:], masks[:])

            # ---- Phase B+C: attention per chunk, with the previous chunk's
            # wo-projection chains interleaved into the jt loop so the
            # in-order PE never sits on PSUM slot recycling ----
            with (
                tc.tile_pool(name="post", bufs=1) as post,
                tc.tile_pool(name="sml", bufs=2) as sml,
                tc.tile_pool(name="otp", bufs=2) as otp,
                tc.tile_pool(name="outp", bufs=4) as outp,
                tc.tile_pool(name="pso", bufs=4, space="PSUM") as pso4,
                tc.tile_pool(name="psd2", bufs=1, space="PSUM") as ps1,
            ):
                # wo tiles land in the freed xw region; their dma_starts ride
                # the sync ring (idle until the first out-store ~15us later)
                # so the slot-free wait never blocks the scalar exp stream
                wosb = []
                for et in range(NH):
                    wt_ = post.tile([128, D], DT, tag=f"wos{et}")
                    nc.sync.dma_start(wt_[:], wo[et * 128:(et + 1) * 128, :])
                    wosb.append(wt_)

                pc_n = [0]

                def emit_pc_chain(c0, lt, oc, otc0, alternate=False):
                    if alternate and pc_n[0] % 2 == 0:
                        ps = ps3.tile([128, TCH], F32, tag="ps_s")
                    else:
                        ps = pso4.tile([128, TCH], F32, tag="ps_o")
                    for h2 in range(NH):
                        nc.tensor.matmul(
                            ps[:],
                            otc0[:, h2, lt * 128:(lt + 1) * 128],
                            wosb[h2][:, oc * TCH:(oc + 1) * TCH],
                            start=(h2 == 0),
                            stop=(h2 == NH - 1),
                        )
                    ost = outp.tile([128, TCH], BF, tag="ost")
                    pc_n[0] += 1
                    if alternate and pc_n[0] % 2 == 0:
                        nc.scalar.copy(ost[:], ps[:])
                    else:
                        nc.vector.tensor_copy(ost[:], ps[:])
                    deng = nc.scalar if (alternate and pc_n[0] % 2 == 0) \
                        else nc.sync
                    deng.dma_start(
                        out[(4 * c0 + lt) * 128:(4 * c0 + lt + 1) * 128,
                            oc * TCH:(oc + 1) * TCH],
                        ost[:],
                    )

                pending = []
                fin_q = []
                for c in (0, 1, 2, 3):
                    otc = otp.tile([128, NH, TCH], DT, tag="ot")
                    njt = 4 * (c + 1) if causal else NTT
                    steps_total = NH * njt
                    spacing = max(1, steps_total // len(pending)) if pending else 0
                    step = 0
                    for h in range(NH):
                        pso = pso4.tile([128, TCH], F32, tag="ps_o")
                        psd = ps1.tile([1, TCH], F32, tag="ps_d")

                        def emit_pss(jt):
                            # diagonal tiles: queries < 128*qd are fully
                            # masked for this key tile -> narrow the moving
                            # operand and all downstream elementwise work
                            qd = jt - 4 * c
                            o = qd * 128 if (causal and qd > 0) else 0
                            pss = ps3.tile([128, TCH], F32, tag="ps_s")
                            nc.tensor.matmul(
                                pss[:, o:],
                                qkt[:, 2 * h, jt * 128:(jt + 1) * 128],
                                qkt[:, 2 * h + 1, c * TCH + o:(c + 1) * TCH],
                                start=True,
                                stop=True,
                            )
                            pt = work.tile([128, TCH], DT, tag="pt")
                            nc.scalar.activation(pt[:, o:], pss[:, o:], EXP,
                                                 scale=scale)
                            if causal:
                                if qd >= 0:
                                    # the mask only zeroes the 128-wide
                                    # strip at the diagonal (keys span 128
                                    # rows), so the mul covers just that strip
                                    nc.vector.tensor_mul(
                                        pt[:, o:o + 128], pt[:, o:o + 128],
                                        msb[:, qd * TCH + o:qd * TCH + o + 128])
                            else:
                                mt = work.tile([128, TCH], DT, tag="mt")
                                nc.sync.dma_start(
                                    mt[:],
                                    maskT[jt * 128:(jt + 1) * 128,
                                          c * TCH:(c + 1) * TCH],
                                )
                                nc.vector.tensor_mul(pt[:], pt[:], mt[:])
                            return pt, o

                        pts = {}
                        jt0 = 0
                        while causal and (c, h, jt0) in pre_pts:
                            pts[jt0] = pre_pts[(c, h, jt0)]
                            jt0 += 1
                        for jt in range(jt0, min(3, njt)):
                            pts[jt] = emit_pss(jt)
                        prev_pt = prev_off = None
                        dpend = []   # pair sums awaiting their ones-matmul
                        dn = [0]

                        def flush_den(stop, dp_=None, psd_=None, dn_=None):
                            # bind per-head state via defaults at call sites:
                            # the loop rebinds dpend/psd/dn each head, so a
                            # deferred call must not resolve them by closure
                            dp_ = dpend if dp_ is None else dp_
                            psd_ = psd if psd_ is None else psd_
                            dn_ = dn if dn_ is None else dn_
                            fpt, foff, _ = dp_.pop(0)
                            nc.tensor.matmul(
                                psd_[:, foff:],
                                ones[:, 0:1],
                                fpt[:, foff:],
                                start=(dn_[0] == 0),
                                stop=stop,
                            )
                            dn_[0] += 1

                        for jt in range(njt):
                            if jt + 3 < njt and jt + 3 not in pts:
                                pts[jt + 3] = emit_pss(jt + 3)
                            pt, o = pts.pop(jt)
                            nc.tensor.matmul(
                                pso[:, o:],
                                vsb[:, jt, h * HD:(h + 1) * HD],
                                pt[:, o:],
                                start=(jt == 0),
                                stop=(jt == njt - 1),
                            )
                            if jt == 1 and fin_q:
                                fin_q.pop(0)()
                            # denominator: sum pt pairs in place on DVE (the
                            # odd tile's valid region is a suffix of the even
                            # tile's) and run the ones-matmul one pair late so
                            # the DVE add latency never stalls the PE
                            if jt % 2 == 0:
                                prev_pt, prev_off = pt, o
                            else:
                                nc.vector.tensor_add(prev_pt[:, o:],
                                                     prev_pt[:, o:], pt[:, o:])
                                if (dpend and dpend[0][1] <= prev_off
                                        and jt != njt - 1):
                                    # absorb into the head's running entry on
                                    # DVE: each absorb kills one ones-matmul;
                                    # the last pair stays separate so the
                                    # flush never waits on a fresh add
                                    e = dpend[0][0]
                                    nc.vector.tensor_add(
                                        e[:, prev_off:], e[:, prev_off:],
                                        prev_pt[:, prev_off:])
                                else:
                                    dpend.append([prev_pt, prev_off, False])
                            step += 1
                            if pending and spacing and step % spacing == 0:
                                emit_pc_chain(*pending.pop(0))
                        def finalize(dp=dpend, fl=flush_den, ps_o=pso,
                                     ps_d=psd, oc_=otc, h_=h, dn_c=dn):
                            while dp:
                                fl(stop=(len(dp) == 1), dp_=dp, psd_=ps_d,
                                   dn_=dn_c)
                            # 1/d via approx reciprocal (~18 bits, beyond the
                            # bf16 pipeline); exact recip costs 3.3us
                            drc = sml.tile([1, TCH], F32, tag="drc")
                            nc.vector.reciprocal_approx_fast(drc[:], ps_d[:])
                            bc = sml.tile([128, TCH], F32, tag="bc")
                            nc.gpsimd.partition_broadcast(bc[:], drc[:])
                            nc.vector.tensor_mul(oc_[:, h_, :], ps_o[:], bc[:])
                        fin_q.append(finalize)
                    while pending:
                        emit_pc_chain(*pending.pop(0))
                    pending = [(c, lt, oc, otc)
                               for lt in range(4) for oc in range(NCH)]
                while fin_q:
                    fin_q.pop(0)()
                # tail drain: alternate the PSUM->SBUF copies between DVE and
                # ScalarE so slot recycling isn't single-engine-latency-bound
                for chain in pending:
                    emit_pc_chain(*chain, alternate=True)
    nc.compile()
    return nc


def _get_built(causal: bool):
    if causal not in _BUILT:
        _BUILT[causal] = _build(causal)
    return _BUILT[causal]


def _diag_masks():
    # masks[jl, q*TCH + ii] = 1 if key (128*q + jl) <= query ii in the chunk
    q = np.arange(4)[:, None, None]
    jl = np.arange(128)[None, :, None]
    ii = np.arange(TCH)[None, None, :]
    m = (ii >= 128 * q + jl).astype(np.float32)        # [4, 128, TCH]
    return np.ascontiguousarray(m.transpose(1, 0, 2).reshape(128, 4 * TCH))


def kernel(x, mask, wqkv, wo):
    global LAST_RESULTS
    from concourse.bass_utils import run_bass_kernel_spmd

    if BF16:
        import ml_dtypes
        sdt = ml_dtypes.bfloat16
    else:
        sdt = np.float32

    x = np.ascontiguousarray(np.asarray(x, dtype=np.float32))
    wqkv = np.asarray(wqkv, dtype=np.float32)
    wo_f = np.ascontiguousarray(np.asarray(wo, dtype=np.float32).astype(sdt))
    mask_np = np.asarray(mask).reshape(T, T).astype(bool)
    causal = bool(np.array_equal(mask_np, np.tril(np.ones((T, T), dtype=bool))))

    nc = _get_built(causal)
    masks_arr = _diag_masks().astype(sdt)
    maskT = None
    if not causal:
        maskT = np.ascontiguousarray(mask_np.T.astype(sdt))

    in_maps = []
    for core in range(NCORES):
        b, g = divmod(core, NH)
        xT = np.ascontiguousarray(x[b].T.astype(sdt))
        wq = wqkv[:, 0 * H * HD + g * NH * HD:0 * H * HD + (g + 1) * NH * HD]
        wk = wqkv[:, 1 * H * HD + g * NH * HD:1 * H * HD + (g + 1) * NH * HD]
        wv = wqkv[:, 2 * H * HD + g * NH * HD:2 * H * HD + (g + 1) * NH * HD]
        wqkv_g = np.ascontiguousarray(
            np.concatenate([wq, wk, wv], axis=1).astype(sdt))
        wo_g = np.ascontiguousarray(wo_f[g * NH * HD:(g + 1) * NH * HD, :])
        m = {"xT": xT, "wqkv": wqkv_g, "wo": wo_g, "masks": masks_arr}
        if maskT is not None:
            m["maskT"] = maskT
        in_maps.append(m)

    trace = os.environ.get("ATTN_TRACE", "") not in ("", "0")
    res = run_bass_kernel_spmd(nc, in_maps, core_ids=list(range(NCORES)),
                               trace=trace)
    LAST_RESULTS = res

    acc = np.zeros((B, T, D), dtype=np.float64)
    for core in range(NCORES):
        b = core // NH
        acc[b] += res.results[core]["out"].astype(np.float64)
    return acc.astype(np.float32)
